# revision 12
# baseline (speedup 1.0000x reference)
"""Trainium2 Bass kernel for the 7-member ensemble dynamics model MLP.

Contract: kernel(**inputs) takes the FULL (unsharded) numpy inputs and
returns the full outputs (next_states [7,32768,32], rewards [7,32768,1]).
Internally the batch axis is sharded 8 ways across the NeuronCores; every
core runs all 7 ensemble members on its 4096-row batch shard.

Dataflow per core:
  - x^T [40, 4096] is built once via PE transposes of [128,40] chunks.
  - Layers 1-4 run feature-major: h^T[l] = silu(W_l^T @ h^T[l-1] + b_l),
    with the 200-wide feature dim split into partition chunks of 128+72.
    Weights (stored [in,out]) are the stationary matmul operand directly.
  - Layer 5 flips to batch-major with zero transposes: lhsT = h4^T chunk
    [feat, 128batch] (stationary), rhs = W5 chunk [feat, 66] (moving)
    -> psum [128batch, 66].
  - Matmul operands are float32r (fp32 storage, reduced-precision PE
    path): 1 cycle/row instead of fp32's 4. PSUM accumulation is fp32.
  - The double-softplus logvar clamp collapses algebraically to
        std = sqrt(exp(min_lv) + exp(max_lv) * sigmoid(logvar - max_lv))
    which is exact in real arithmetic, so the epilogue needs only two
    ACT transcendentals (Sigmoid, Sqrt) per member.
  - Ensemble members are processed in groups of 3/2/2; each group's
    epilogue ACT ops are phase-batched (all sigmoids, then all sqrts)
    and explicit dep edges keep the ACT instruction stream grouped so
    activation-table reloads stay rare.
"""

import numpy as np

S = 32
A = 8
D = S + A          # 40
H = 200
HA = 128           # first feature chunk
HB = H - HA        # 72
E = 7
B = 32768
SP = S + 1         # 33
OUT = 2 * SP       # 66
N_CORES = 8
B_CORE = B // N_CORES  # 4096
GROUPS = ((0, 1, 2), (3, 4), (5, 6))

_COMPILED = {}


def _build(nc, b_core, n_e, sim_safe=False, use_f32r=False):
    # NOTE: use_f32r=True (reduced-precision fp32 matmul path, 4x faster on
    # the PE) currently hard-crashes the exec unit (NRT status 101); keep
    # the exact-fp32 path.
    import concourse.tile as tile
    from concourse import mybir
    from concourse.masks import make_identity
    from bass_rust import add_dep_helper

    F32 = mybir.dt.float32
    F32R = mybir.dt.float32r if use_f32r else F32
    AF = mybir.ActivationFunctionType

    T = b_core // 512      # 512-wide matmul tiles
    CT = 4                 # 128-chunks per 512 tile
    C_ALL = b_core // 128  # 128-chunks per core

    state = nc.dram_tensor("state", [b_core, S], F32, kind="ExternalInput")
    action = nc.dram_tensor("action", [b_core, A], F32, kind="ExternalInput")
    noise = nc.dram_tensor("noise", [n_e, b_core, SP], F32, kind="ExternalInput")
    W1 = nc.dram_tensor("W1", [n_e, D, H], F32, kind="ExternalInput")
    b1 = nc.dram_tensor("b1", [n_e, H], F32, kind="ExternalInput")
    W2 = nc.dram_tensor("W2", [n_e, H, H], F32, kind="ExternalInput")
    b2 = nc.dram_tensor("b2", [n_e, H], F32, kind="ExternalInput")
    W3 = nc.dram_tensor("W3", [n_e, H, H], F32, kind="ExternalInput")
    b3 = nc.dram_tensor("b3", [n_e, H], F32, kind="ExternalInput")
    W4 = nc.dram_tensor("W4", [n_e, H, H], F32, kind="ExternalInput")
    b4 = nc.dram_tensor("b4", [n_e, H], F32, kind="ExternalInput")
    W5 = nc.dram_tensor("W5", [n_e, H, OUT], F32, kind="ExternalInput")
    b5 = nc.dram_tensor("b5", [n_e, OUT], F32, kind="ExternalInput")
    max_logvar = nc.dram_tensor("max_logvar", [1, SP], F32, kind="ExternalInput")
    min_logvar = nc.dram_tensor("min_logvar", [1, SP], F32, kind="ExternalInput")
    next_states = nc.dram_tensor(
        "next_states", [n_e, b_core, S], F32, kind="ExternalOutput"
    )
    rewards = nc.dram_tensor("rewards", [n_e, b_core, 1], F32, kind="ExternalOutput")

    Ws = [(W1, b1), (W2, b2), (W3, b3), (W4, b4)]
    groups = [tuple(e for e in g if e < n_e) for g in GROUPS]
    groups = [g for g in groups if g]
    if n_e > 7:
        groups = [tuple(range(i, min(i + 3, n_e))) for i in range(0, n_e, 3)]

    with tile.TileContext(nc) as tc:
        with (
            tc.tile_pool(name="const", bufs=1) as const,
            tc.tile_pool(name="xc", bufs=4) as xcp,
            tc.tile_pool(name="w", bufs=2) as wp,
            tc.tile_pool(name="h", bufs=2) as hp,
            tc.tile_pool(name="o5p", bufs=3) as o5p,
            tc.tile_pool(name="nzp", bufs=3) as nzp,
            tc.tile_pool(name="ep", bufs=2) as ep,
            tc.tile_pool(name="pm0", bufs=3, space="PSUM") as pm0,
            tc.tile_pool(name="pm1", bufs=2, space="PSUM") as pm1,
            tc.tile_pool(name="pl5", bufs=2, space="PSUM") as pl5,
            tc.tile_pool(name="ptr", bufs=1, space="PSUM") as ptr,
        ):
            # ---- constants ----
            ident = const.tile([128, 128], F32, name="ident")
            make_identity(nc, ident[:])

            a_rep = const.tile([128, C_ALL, SP], F32, name="a_rep")
            nc.sync.dma_start(
                a_rep[:],
                max_logvar[0:1, :].unsqueeze(1).broadcast_to([128, C_ALL, SP]),
            )
            c_rep = const.tile([128, C_ALL, SP], F32, name="c_rep")
            nc.sync.dma_start(
                c_rep[:],
                min_logvar[0:1, :].unsqueeze(1).broadcast_to([128, C_ALL, SP]),
            )
            # ex_rep = exp(max_logvar), en_rep = exp(min_logvar), broadcast
            ex_rep = const.tile([128, C_ALL, SP], F32, name="ex_rep")
            nc.scalar.activation(ex_rep[:], a_rep[:], AF.Exp)
            en_rep = const.tile([128, C_ALL, SP], F32, name="en_rep")
            nc.scalar.activation(en_rep[:], c_rep[:], AF.Exp)

            state_b = const.tile([128, C_ALL, S], F32, name="state_b")
            nc.sync.dma_start(
                state_b[:], state[:].rearrange("(c p) j -> p c j", p=128)
            )

            # ---- x^T [40, b_core] via PE transposes ----
            xT = const.tile([D, b_core], F32R, name="xT")
            for c in range(C_ALL):
                xcat = xcp.tile([128, D], F32, tag="xcat", name=f"xcat{c}")
                nc.sync.dma_start(xcat[:, 0:S], state[c * 128 : (c + 1) * 128, :])
                nc.sync.dma_start(xcat[:, S:D], action[c * 128 : (c + 1) * 128, :])
                pt = ptr.tile([D, 128], F32, tag="ptr", name=f"pt{c}")
                nc.tensor.transpose(pt[:], xcat[:], ident[:])
                nc.vector.tensor_copy(xT[:, c * 128 : (c + 1) * 128], pt[:])

            # ACT-table grouping: silu ops of group g+1 depend on the last
            # epilogue ACT ops (sqrts) of group g so the ACT stream stays
            # [silu block][sigmoid block][sqrt block] per group.
            prev_group_acts = []

            def emit_silu(out, ps, bias, name):
                if not sim_safe:
                    insts = [nc.scalar.activation(out, ps, AF.Silu, bias=bias)]
                else:
                    # CoreSim has no Silu; equivalent decomposition
                    z = hp.tile(list(out.shape), F32, tag="simz", bufs=4, name=f"z{name}")
                    i1 = nc.scalar.activation(z[:], ps, AF.Identity, bias=bias)
                    g = hp.tile(list(out.shape), F32, tag="simg", bufs=4, name=f"g{name}")
                    i2 = nc.scalar.activation(g[:], ps, AF.Sigmoid, bias=bias)
                    nc.vector.tensor_mul(out, z[:], g[:])
                    insts = [i1, i2]
                for i in insts:
                    for p in prev_group_acts:
                        add_dep_helper(i.ins, p.ins, sync=False,
                                       reason="act-table-grouping")

            def round_tile(dst_shape, tag, name, src_ap):
                """DMA fp32 from DRAM then round to f32r via DVE copy."""
                if not use_f32r:
                    t = wp.tile(dst_shape, F32, tag=tag, name=name)
                    nc.sync.dma_start(t[:], src_ap)
                    return t
                stg = wp.tile([dst_shape[0], dst_shape[1]], F32, tag=f"{tag}_s",
                              name=f"{name}_s")
                nc.sync.dma_start(stg[:], src_ap)
                t = wp.tile(dst_shape, F32R, tag=tag, name=name)
                nc.vector.tensor_copy(t[:], stg[:])
                return t

            def run_member(e):
                """Weights + 5 layers + psum drain for member e. Returns
                (o5, nz) for the deferred epilogue."""
                w1t = round_tile([D, H], "w1", f"w1_{e}", W1[e])
                wts = [w1t]
                bts = []
                for li, (Wl, bl) in enumerate(Ws):
                    if li > 0:
                        wa = round_tile([HA, H], f"w{li + 1}a", f"w{li + 1}a_{e}",
                                        Wl[e, 0:HA, :])
                        wb = round_tile([HB, H], f"w{li + 1}b", f"w{li + 1}b_{e}",
                                        Wl[e, HA:H, :])
                        wts.append((wa, wb))
                    ba = wp.tile([HA, 1], F32, tag=f"b{li + 1}a", name=f"b{li + 1}a_{e}")
                    nc.sync.dma_start(ba[:], bl[e : e + 1, 0:HA].rearrange("o p -> p o"))
                    bb = wp.tile([HB, 1], F32, tag=f"b{li + 1}b", name=f"b{li + 1}b_{e}")
                    nc.sync.dma_start(bb[:], bl[e : e + 1, HA:H].rearrange("o p -> p o"))
                    bts.append((ba, bb))
                w5a = round_tile([HA, OUT], "w5a", f"w5a_{e}", W5[e, 0:HA, :])
                w5b = round_tile([HB, OUT], "w5b", f"w5b_{e}", W5[e, HA:H, :])
                b5r = wp.tile([128, CT, OUT], F32, tag="b5r", name=f"b5r_{e}")
                nc.sync.dma_start(
                    b5r[:], b5[e : e + 1, :].unsqueeze(1).broadcast_to([128, CT, OUT])
                )

                nz = nzp.tile([128, C_ALL, SP], F32, tag="nz", name=f"nz_{e}")
                nc.sync.dma_start(
                    nz[:], noise[e].rearrange("(c p) j -> p c j", p=128)
                )

                o5 = o5p.tile([128, C_ALL, OUT], F32, tag="o5", name=f"o5_{e}")

                for t in range(T):
                    tsl = slice(t * 512, (t + 1) * 512)
                    # L1: K=40
                    ps = pm0.tile([HA, 512], F32, tag="pm0", name=f"ps1a_{e}_{t}")
                    nc.tensor.matmul(ps[:], w1t[:, 0:HA], xT[:, tsl], start=True, stop=True)
                    ha_t = hp.tile([HA, 512], F32R, tag="ha", bufs=10, name=f"h1a_{e}_{t}")
                    emit_silu(ha_t[:], ps[:], bts[0][0][:], f"1a{e}_{t}")
                    ps = pm1.tile([HB, 512], F32, tag="pm1", name=f"ps1b_{e}_{t}")
                    nc.tensor.matmul(ps[:], w1t[:, HA:H], xT[:, tsl], start=True, stop=True)
                    hb_t = hp.tile([HB, 512], F32R, tag="hb", bufs=10, name=f"h1b_{e}_{t}")
                    emit_silu(hb_t[:], ps[:], bts[0][1][:], f"1b{e}_{t}")

                    # L2-L4: K=200 in two chunks
                    for li in range(1, 4):
                        wa, wb = wts[li]
                        ba, bb = bts[li]
                        ps = pm0.tile([HA, 512], F32, tag="pm0", name=f"psa_{e}_{t}_{li}")
                        nc.tensor.matmul(ps[:], wa[:, 0:HA], ha_t[:], start=True, stop=False)
                        nc.tensor.matmul(ps[:], wb[:, 0:HA], hb_t[:], start=False, stop=True)
                        nha = hp.tile([HA, 512], F32R, tag="ha", bufs=10, name=f"ha_{e}_{t}_{li}")
                        emit_silu(nha[:], ps[:], ba[:], f"a{e}_{t}_{li}")
                        ps = pm1.tile([HB, 512], F32, tag="pm1", name=f"psb_{e}_{t}_{li}")
                        nc.tensor.matmul(ps[:], wa[:, HA:H], ha_t[:], start=True, stop=False)
                        nc.tensor.matmul(ps[:], wb[:, HA:H], hb_t[:], start=False, stop=True)
                        nhb = hp.tile([HB, 512], F32R, tag="hb", bufs=10, name=f"hb_{e}_{t}_{li}")
                        emit_silu(nhb[:], ps[:], bb[:], f"b{e}_{t}_{li}")
                        ha_t, hb_t = nha, nhb

                    # L5: batch-major out, activations stationary
                    ps5 = pl5.tile([128, CT, OUT], F32, tag="pl5", name=f"ps5_{e}_{t}")
                    for c in range(CT):
                        csl = slice(c * 128, (c + 1) * 128)
                        nc.tensor.matmul(
                            ps5[:, c, :], ha_t[:, csl], w5a[:], start=True, stop=False
                        )
                        nc.tensor.matmul(
                            ps5[:, c, :], hb_t[:, csl], w5b[:], start=False, stop=True
                        )
                    nc.vector.tensor_add(
                        o5[:, t * CT : (t + 1) * CT, :], ps5[:], b5r[:]
                    )
                return o5, nz

            # ---- grouped pipeline ----
            for g in groups:
                held = [run_member(e) for e in g]
                # phase A: y = logvar - max_lv  (DVE)
                ys = []
                for (o5, _), e in zip(held, g):
                    y = ep.tile([128, C_ALL, SP], F32, tag="ep", bufs=8, name=f"y_{e}")
                    nc.vector.tensor_sub(y[:], o5[:, :, SP:OUT], a_rep[:])
                    ys.append(y)
                # phase B: g = sigmoid(y)  (ACT, sigmoid table)
                sgs, sg_insts = [], []
                for y, e in zip(ys, g):
                    sg = ep.tile([128, C_ALL, SP], F32, tag="ep", bufs=8, name=f"sg_{e}")
                    i = nc.scalar.activation(sg[:], y[:], AF.Sigmoid)
                    for p in prev_group_acts:
                        add_dep_helper(i.ins, p.ins, sync=False, reason="act-grouping")
                    sg_insts.append(i)
                    sgs.append(sg)
                # phase C: v = en + ex * g  (DVE)
                vs = []
                for sg, e in zip(sgs, g):
                    v = ep.tile([128, C_ALL, SP], F32, tag="ep", bufs=8, name=f"v_{e}")
                    nc.vector.tensor_mul(v[:], sg[:], ex_rep[:])
                    v2 = ep.tile([128, C_ALL, SP], F32, tag="ep", bufs=8, name=f"v2_{e}")
                    nc.vector.tensor_add(v2[:], v[:], en_rep[:])
                    vs.append(v2)
                # phase D: std = sqrt(v)  (ACT, sqrt table)
                stds, std_insts = [], []
                for v2, e in zip(vs, g):
                    std = ep.tile([128, C_ALL, SP], F32, tag="ep", bufs=8, name=f"std_{e}")
                    i = nc.scalar.activation(std[:], v2[:], AF.Sqrt)
                    for p in sg_insts:
                        add_dep_helper(i.ins, p.ins, sync=False, reason="act-grouping")
                    std_insts.append(i)
                    stds.append(std)
                prev_group_acts = std_insts
                # phase E: samples, outputs  (DVE + DMA)
                for (o5, nz), std, e in zip(held, stds, g):
                    mean = o5[:, :, 0:SP]
                    sn = ep.tile([128, C_ALL, SP], F32, tag="ep", bufs=8, name=f"sn_{e}")
                    nc.vector.tensor_mul(sn[:], std[:], nz[:])
                    ns1 = ep.tile([128, C_ALL, S], F32, tag="ns", bufs=4, name=f"ns1_{e}")
                    nc.vector.tensor_add(ns1[:], mean[:, :, 0:S], sn[:, :, 0:S])
                    ns2 = ep.tile([128, C_ALL, S], F32, tag="ns", bufs=4, name=f"ns2_{e}")
                    nc.vector.tensor_add(ns2[:], ns1[:], state_b[:])
                    nc.sync.dma_start(
                        next_states[e].rearrange("(c p) j -> p c j", p=128), ns2[:]
                    )
                    rw = ep.tile([128, C_ALL, 1], F32, tag="rw", bufs=2, name=f"rw_{e}")
                    nc.vector.tensor_add(rw[:], mean[:, :, S:SP], sn[:, :, S:SP])
                    nc.sync.dma_start(
                        rewards[e].rearrange("(c p) j -> p c j", p=128), rw[:]
                    )
    return nc


def _get_compiled(b_core=B_CORE, n_e=E):
    key = (b_core, n_e)
    if key not in _COMPILED:
        from concourse import bacc

        nc = bacc.Bacc(
            "TRN2", target_bir_lowering=False, debug=False, num_devices=N_CORES
        )
        _build(nc, b_core, n_e)
        nc.compile()
        _COMPILED[key] = nc
    return _COMPILED[key]


def kernel(state, action, noise, W1, b1, W2, b2, W3, b3, W4, b4, W5, b5,
           max_logvar, min_logvar):
    from concourse.bass_utils import run_bass_kernel_spmd

    nc = _get_compiled()
    full = dict(W1=W1, b1=b1, W2=W2, b2=b2, W3=W3, b3=b3, W4=W4, b4=b4,
                W5=W5, b5=b5, max_logvar=max_logvar, min_logvar=min_logvar)
    full = {k: np.ascontiguousarray(np.asarray(v, np.float32)) for k, v in full.items()}
    in_maps = []
    for c in range(N_CORES):
        bsl = slice(c * B_CORE, (c + 1) * B_CORE)
        m = dict(full)
        m["state"] = np.ascontiguousarray(np.asarray(state[bsl], np.float32))
        m["action"] = np.ascontiguousarray(np.asarray(action[bsl], np.float32))
        m["noise"] = np.ascontiguousarray(np.asarray(noise[:, bsl, :], np.float32))
        in_maps.append(m)

    res = run_bass_kernel_spmd(nc, in_maps, core_ids=list(range(N_CORES)))
    next_states = np.concatenate(
        [res.results[c]["next_states"] for c in range(N_CORES)], axis=1
    )
    rewards = np.concatenate(
        [res.results[c]["rewards"] for c in range(N_CORES)], axis=1
    )
    return next_states, rewards


# revision 13
# speedup vs baseline: 1.7736x; 1.7736x over previous
"""Trainium2 Bass kernel for the 7-member ensemble dynamics model MLP.

Contract: kernel(**inputs) takes the FULL (unsharded) numpy inputs and
returns the full outputs (next_states [7,32768,32], rewards [7,32768,1]).
Internally the batch axis is sharded 8 ways across the NeuronCores; every
core runs all 7 ensemble members on its 4096-row batch shard.

Dataflow per core:
  - x^T [40, 4096] is built once via PE transposes of [128,40] chunks.
  - Layers 1-4 run feature-major: h^T[l] = silu(W_l^T @ h^T[l-1] + b_l),
    with the 200-wide feature dim split into partition chunks of 128+72.
    Weights (stored [in,out]) are the stationary matmul operand directly.
  - Layer 5 flips to batch-major with zero transposes: lhsT = h4^T chunk
    [feat, 128batch] (stationary), rhs = W5 chunk [feat, 66] (moving)
    -> psum [128batch, 66].
  - Matmul operands are float32r (fp32 storage, reduced-precision PE
    path): 1 cycle/row instead of fp32's 4. PSUM accumulation is fp32.
  - The double-softplus logvar clamp collapses algebraically to
        std = sqrt(exp(min_lv) + exp(max_lv) * sigmoid(logvar - max_lv))
    which is exact in real arithmetic, so the epilogue needs only two
    ACT transcendentals (Sigmoid, Sqrt) per member.
  - Ensemble members are processed in groups of 3/2/2; each group's
    epilogue ACT ops are phase-batched (all sigmoids, then all sqrts)
    and explicit dep edges keep the ACT instruction stream grouped so
    activation-table reloads stay rare.
"""

import numpy as np

S = 32
A = 8
D = S + A          # 40
H = 200
HA = 128           # first feature chunk
HB = H - HA        # 72
E = 7
B = 32768
SP = S + 1         # 33
OUT = 2 * SP       # 66
N_CORES = 8
B_CORE = B // N_CORES  # 4096
GROUPS = ((0, 1, 2), (3, 4), (5, 6))
USE_F32R = False

_COMPILED = {}


def _build(nc, b_core, n_e, sim_safe=False, use_f32r=None):
    # use_f32r: reduced-precision fp32 matmul path (float32r), 4x faster on
    # the PE. All transposes are ordered strictly before the first f32r
    # matmul (PE mode-transition hazard suspected in the NRT-101 crash).
    if use_f32r is None:
        use_f32r = USE_F32R
    import concourse.tile as tile
    from concourse import mybir
    from concourse.masks import make_identity
    from bass_rust import add_dep_helper

    F32 = mybir.dt.float32
    F32R = mybir.dt.float32r if use_f32r else F32
    AF = mybir.ActivationFunctionType

    T = b_core // 512      # 512-wide matmul tiles
    CT = 4                 # 128-chunks per 512 tile
    C_ALL = b_core // 128  # 128-chunks per core

    state = nc.dram_tensor("state", [b_core, S], F32, kind="ExternalInput")
    action = nc.dram_tensor("action", [b_core, A], F32, kind="ExternalInput")
    noise = nc.dram_tensor("noise", [n_e, b_core, SP], F32, kind="ExternalInput")
    W1 = nc.dram_tensor("W1", [n_e, D, H], F32, kind="ExternalInput")
    b1 = nc.dram_tensor("b1", [n_e, H], F32, kind="ExternalInput")
    W2 = nc.dram_tensor("W2", [n_e, H, H], F32, kind="ExternalInput")
    b2 = nc.dram_tensor("b2", [n_e, H], F32, kind="ExternalInput")
    W3 = nc.dram_tensor("W3", [n_e, H, H], F32, kind="ExternalInput")
    b3 = nc.dram_tensor("b3", [n_e, H], F32, kind="ExternalInput")
    W4 = nc.dram_tensor("W4", [n_e, H, H], F32, kind="ExternalInput")
    b4 = nc.dram_tensor("b4", [n_e, H], F32, kind="ExternalInput")
    W5 = nc.dram_tensor("W5", [n_e, H, OUT], F32, kind="ExternalInput")
    b5 = nc.dram_tensor("b5", [n_e, OUT], F32, kind="ExternalInput")
    max_logvar = nc.dram_tensor("max_logvar", [1, SP], F32, kind="ExternalInput")
    min_logvar = nc.dram_tensor("min_logvar", [1, SP], F32, kind="ExternalInput")
    next_states = nc.dram_tensor(
        "next_states", [n_e, b_core, S], F32, kind="ExternalOutput"
    )
    rewards = nc.dram_tensor("rewards", [n_e, b_core, 1], F32, kind="ExternalOutput")

    Ws = [(W1, b1), (W2, b2), (W3, b3), (W4, b4)]
    groups = [tuple(e for e in g if e < n_e) for g in GROUPS]
    groups = [g for g in groups if g]
    if n_e > 7:
        groups = [tuple(range(i, min(i + 3, n_e))) for i in range(0, n_e, 3)]

    with tile.TileContext(nc) as tc:
        with (
            tc.tile_pool(name="const", bufs=1) as const,
            tc.tile_pool(name="xc", bufs=4) as xcp,
            tc.tile_pool(name="w", bufs=2) as wp,
            tc.tile_pool(name="h", bufs=2) as hp,
            tc.tile_pool(name="o5p", bufs=3) as o5p,
            tc.tile_pool(name="nzp", bufs=3) as nzp,
            tc.tile_pool(name="ep", bufs=2) as ep,
            tc.tile_pool(name="pm0", bufs=3, space="PSUM") as pm0,
            tc.tile_pool(name="pm1", bufs=2, space="PSUM") as pm1,
            tc.tile_pool(name="pl5", bufs=2, space="PSUM") as pl5,
            tc.tile_pool(name="ptr", bufs=1, space="PSUM") as ptr,
        ):
            # ---- constants ----
            ident = const.tile([128, 128], F32, name="ident")
            make_identity(nc, ident[:])

            a_rep = const.tile([128, C_ALL, SP], F32, name="a_rep")
            nc.sync.dma_start(
                a_rep[:],
                max_logvar[0:1, :].unsqueeze(1).broadcast_to([128, C_ALL, SP]),
            )
            c_rep = const.tile([128, C_ALL, SP], F32, name="c_rep")
            nc.sync.dma_start(
                c_rep[:],
                min_logvar[0:1, :].unsqueeze(1).broadcast_to([128, C_ALL, SP]),
            )
            # ex_rep = exp(max_logvar), en_rep = exp(min_logvar), broadcast
            ex_rep = const.tile([128, C_ALL, SP], F32, name="ex_rep")
            nc.scalar.activation(ex_rep[:], a_rep[:], AF.Exp)
            en_rep = const.tile([128, C_ALL, SP], F32, name="en_rep")
            nc.scalar.activation(en_rep[:], c_rep[:], AF.Exp)

            state_b = const.tile([128, C_ALL, S], F32, name="state_b")
            nc.sync.dma_start(
                state_b[:], state[:].rearrange("(c p) j -> p c j", p=128)
            )

            # ---- x^T [40, b_core] via PE transposes ----
            tr_insts = []
            xT = const.tile([D, b_core], F32R, name="xT")
            for c in range(C_ALL):
                xcat = xcp.tile([128, D], F32, tag="xcat", name=f"xcat{c}")
                nc.sync.dma_start(xcat[:, 0:S], state[c * 128 : (c + 1) * 128, :])
                nc.sync.dma_start(xcat[:, S:D], action[c * 128 : (c + 1) * 128, :])
                pt = ptr.tile([D, 128], F32, tag="ptr", name=f"pt{c}")
                tr_insts.append(nc.tensor.transpose(pt[:], xcat[:], ident[:]))
                nc.vector.tensor_copy(xT[:, c * 128 : (c + 1) * 128], pt[:])

            # ACT-table grouping: silu ops of group g+1 depend on the last
            # epilogue ACT ops (sqrts) of group g so the ACT stream stays
            # [silu block][sigmoid block][sqrt block] per group.
            prev_group_acts = []

            def emit_silu(out, ps, bias, name):
                if not sim_safe:
                    insts = [nc.scalar.activation(out, ps, AF.Silu, bias=bias)]
                else:
                    # CoreSim has no Silu; equivalent decomposition
                    z = hp.tile(list(out.shape), F32, tag="simz", bufs=4, name=f"z{name}")
                    i1 = nc.scalar.activation(z[:], ps, AF.Identity, bias=bias)
                    g = hp.tile(list(out.shape), F32, tag="simg", bufs=4, name=f"g{name}")
                    i2 = nc.scalar.activation(g[:], ps, AF.Sigmoid, bias=bias)
                    nc.vector.tensor_mul(out, z[:], g[:])
                    insts = [i1, i2]
                for i in insts:
                    for p in prev_group_acts:
                        add_dep_helper(i.ins, p.ins, sync=False,
                                       reason="act-table-grouping")

            def round_tile(dst_shape, tag, name, src_ap):
                """DMA fp32 from DRAM then round to f32r via DVE copy."""
                if not use_f32r:
                    t = wp.tile(dst_shape, F32, tag=tag, name=name)
                    nc.sync.dma_start(t[:], src_ap)
                    return t
                stg = wp.tile([dst_shape[0], dst_shape[1]], F32, tag=f"{tag}_s",
                              name=f"{name}_s")
                nc.sync.dma_start(stg[:], src_ap)
                t = wp.tile(dst_shape, F32R, tag=tag, name=name)
                nc.vector.tensor_copy(t[:], stg[:])
                return t

            def run_member(e):
                """Weights + 5 layers + psum drain for member e. Returns
                (o5, nz) for the deferred epilogue."""
                w1t = round_tile([D, H], "w1", f"w1_{e}", W1[e])
                wts = [w1t]
                bts = []
                for li, (Wl, bl) in enumerate(Ws):
                    if li > 0:
                        wa = round_tile([HA, H], f"w{li + 1}a", f"w{li + 1}a_{e}",
                                        Wl[e, 0:HA, :])
                        wb = round_tile([HB, H], f"w{li + 1}b", f"w{li + 1}b_{e}",
                                        Wl[e, HA:H, :])
                        wts.append((wa, wb))
                    ba = wp.tile([HA, 1], F32, tag=f"b{li + 1}a", name=f"b{li + 1}a_{e}")
                    nc.sync.dma_start(ba[:], bl[e : e + 1, 0:HA].rearrange("o p -> p o"))
                    bb = wp.tile([HB, 1], F32, tag=f"b{li + 1}b", name=f"b{li + 1}b_{e}")
                    nc.sync.dma_start(bb[:], bl[e : e + 1, HA:H].rearrange("o p -> p o"))
                    bts.append((ba, bb))
                w5a = round_tile([HA, OUT], "w5a", f"w5a_{e}", W5[e, 0:HA, :])
                w5b = round_tile([HB, OUT], "w5b", f"w5b_{e}", W5[e, HA:H, :])
                b5r = wp.tile([128, CT, OUT], F32, tag="b5r", name=f"b5r_{e}")
                nc.sync.dma_start(
                    b5r[:], b5[e : e + 1, :].unsqueeze(1).broadcast_to([128, CT, OUT])
                )

                nz = nzp.tile([128, C_ALL, SP], F32, tag="nz", name=f"nz_{e}")
                nc.sync.dma_start(
                    nz[:], noise[e].rearrange("(c p) j -> p c j", p=128)
                )

                o5 = o5p.tile([128, C_ALL, OUT], F32, tag="o5", name=f"o5_{e}")

                for t in range(T):
                    tsl = slice(t * 512, (t + 1) * 512)
                    # L1: K=40
                    ps = pm0.tile([HA, 512], F32, tag="pm0", name=f"ps1a_{e}_{t}")
                    mm = nc.tensor.matmul(ps[:], w1t[:, 0:HA], xT[:, tsl], start=True, stop=True)
                    if tr_insts:
                        # order all fp32 transpose-mode PE ops before the
                        # first normal matmul (PE is in-order apart from
                        # LDWEIGHTS pull-ahead, so one edge orders the rest)
                        for ti in tr_insts:
                            add_dep_helper(mm.ins, ti.ins, sync=False,
                                           reason="transpose-before-matmul")
                        tr_insts.clear()
                    ha_t = hp.tile([HA, 512], F32R, tag="ha", bufs=10, name=f"h1a_{e}_{t}")
                    emit_silu(ha_t[:], ps[:], bts[0][0][:], f"1a{e}_{t}")
                    ps = pm1.tile([HB, 512], F32, tag="pm1", name=f"ps1b_{e}_{t}")
                    nc.tensor.matmul(ps[:], w1t[:, HA:H], xT[:, tsl], start=True, stop=True)
                    hb_t = hp.tile([HB, 512], F32R, tag="hb", bufs=10, name=f"h1b_{e}_{t}")
                    emit_silu(hb_t[:], ps[:], bts[0][1][:], f"1b{e}_{t}")

                    # L2-L4: K=200 in two chunks
                    for li in range(1, 4):
                        wa, wb = wts[li]
                        ba, bb = bts[li]
                        ps = pm0.tile([HA, 512], F32, tag="pm0", name=f"psa_{e}_{t}_{li}")
                        nc.tensor.matmul(ps[:], wa[:, 0:HA], ha_t[:], start=True, stop=False)
                        nc.tensor.matmul(ps[:], wb[:, 0:HA], hb_t[:], start=False, stop=True)
                        nha = hp.tile([HA, 512], F32R, tag="ha", bufs=10, name=f"ha_{e}_{t}_{li}")
                        emit_silu(nha[:], ps[:], ba[:], f"a{e}_{t}_{li}")
                        ps = pm1.tile([HB, 512], F32, tag="pm1", name=f"psb_{e}_{t}_{li}")
                        nc.tensor.matmul(ps[:], wa[:, HA:H], ha_t[:], start=True, stop=False)
                        nc.tensor.matmul(ps[:], wb[:, HA:H], hb_t[:], start=False, stop=True)
                        nhb = hp.tile([HB, 512], F32R, tag="hb", bufs=10, name=f"hb_{e}_{t}_{li}")
                        emit_silu(nhb[:], ps[:], bb[:], f"b{e}_{t}_{li}")
                        ha_t, hb_t = nha, nhb

                    # L5: batch-major out, activations stationary
                    ps5 = pl5.tile([128, CT, OUT], F32, tag="pl5", name=f"ps5_{e}_{t}")
                    for c in range(CT):
                        csl = slice(c * 128, (c + 1) * 128)
                        nc.tensor.matmul(
                            ps5[:, c, :], ha_t[:, csl], w5a[:], start=True, stop=False
                        )
                        nc.tensor.matmul(
                            ps5[:, c, :], hb_t[:, csl], w5b[:], start=False, stop=True
                        )
                    nc.vector.tensor_add(
                        o5[:, t * CT : (t + 1) * CT, :], ps5[:], b5r[:]
                    )
                return o5, nz

            # ---- grouped pipeline ----
            for g in groups:
                held = [run_member(e) for e in g]
                # phase A: y = logvar - max_lv  (DVE)
                ys = []
                for (o5, _), e in zip(held, g):
                    y = ep.tile([128, C_ALL, SP], F32, tag="ep", bufs=8, name=f"y_{e}")
                    nc.vector.tensor_sub(y[:], o5[:, :, SP:OUT], a_rep[:])
                    ys.append(y)
                # phase B: g = sigmoid(y)  (ACT, sigmoid table)
                sgs, sg_insts = [], []
                for y, e in zip(ys, g):
                    sg = ep.tile([128, C_ALL, SP], F32, tag="ep", bufs=8, name=f"sg_{e}")
                    i = nc.scalar.activation(sg[:], y[:], AF.Sigmoid)
                    for p in prev_group_acts:
                        add_dep_helper(i.ins, p.ins, sync=False, reason="act-grouping")
                    sg_insts.append(i)
                    sgs.append(sg)
                # phase C: v = en + ex * g  (DVE)
                vs = []
                for sg, e in zip(sgs, g):
                    v = ep.tile([128, C_ALL, SP], F32, tag="ep", bufs=8, name=f"v_{e}")
                    nc.vector.tensor_mul(v[:], sg[:], ex_rep[:])
                    v2 = ep.tile([128, C_ALL, SP], F32, tag="ep", bufs=8, name=f"v2_{e}")
                    nc.vector.tensor_add(v2[:], v[:], en_rep[:])
                    vs.append(v2)
                # phase D: std = sqrt(v)  (ACT, sqrt table)
                stds, std_insts = [], []
                for v2, e in zip(vs, g):
                    std = ep.tile([128, C_ALL, SP], F32, tag="ep", bufs=8, name=f"std_{e}")
                    i = nc.scalar.activation(std[:], v2[:], AF.Sqrt)
                    for p in sg_insts:
                        add_dep_helper(i.ins, p.ins, sync=False, reason="act-grouping")
                    std_insts.append(i)
                    stds.append(std)
                prev_group_acts = std_insts
                # phase E: samples, outputs  (DVE + DMA)
                for (o5, nz), std, e in zip(held, stds, g):
                    mean = o5[:, :, 0:SP]
                    sn = ep.tile([128, C_ALL, SP], F32, tag="ep", bufs=8, name=f"sn_{e}")
                    nc.vector.tensor_mul(sn[:], std[:], nz[:])
                    ns1 = ep.tile([128, C_ALL, S], F32, tag="ns", bufs=4, name=f"ns1_{e}")
                    nc.vector.tensor_add(ns1[:], mean[:, :, 0:S], sn[:, :, 0:S])
                    ns2 = ep.tile([128, C_ALL, S], F32, tag="ns", bufs=4, name=f"ns2_{e}")
                    nc.vector.tensor_add(ns2[:], ns1[:], state_b[:])
                    nc.sync.dma_start(
                        next_states[e].rearrange("(c p) j -> p c j", p=128), ns2[:]
                    )
                    rw = ep.tile([128, C_ALL, 1], F32, tag="rw", bufs=2, name=f"rw_{e}")
                    nc.vector.tensor_add(rw[:], mean[:, :, S:SP], sn[:, :, S:SP])
                    nc.sync.dma_start(
                        rewards[e].rearrange("(c p) j -> p c j", p=128), rw[:]
                    )
    return nc


def _get_compiled(b_core=B_CORE, n_e=E):
    key = (b_core, n_e, USE_F32R)
    if key not in _COMPILED:
        from concourse import bacc

        nc = bacc.Bacc(
            "TRN2", target_bir_lowering=False, debug=False, num_devices=N_CORES
        )
        _build(nc, b_core, n_e)
        nc.compile()
        _COMPILED[key] = nc
    return _COMPILED[key]


def kernel(state, action, noise, W1, b1, W2, b2, W3, b3, W4, b4, W5, b5,
           max_logvar, min_logvar):
    from concourse.bass_utils import run_bass_kernel_spmd

    nc = _get_compiled()
    full = dict(W1=W1, b1=b1, W2=W2, b2=b2, W3=W3, b3=b3, W4=W4, b4=b4,
                W5=W5, b5=b5, max_logvar=max_logvar, min_logvar=min_logvar)
    full = {k: np.ascontiguousarray(np.asarray(v, np.float32)) for k, v in full.items()}
    in_maps = []
    for c in range(N_CORES):
        bsl = slice(c * B_CORE, (c + 1) * B_CORE)
        m = dict(full)
        m["state"] = np.ascontiguousarray(np.asarray(state[bsl], np.float32))
        m["action"] = np.ascontiguousarray(np.asarray(action[bsl], np.float32))
        m["noise"] = np.ascontiguousarray(np.asarray(noise[:, bsl, :], np.float32))
        in_maps.append(m)

    res = run_bass_kernel_spmd(nc, in_maps, core_ids=list(range(N_CORES)))
    next_states = np.concatenate(
        [res.results[c]["next_states"] for c in range(N_CORES)], axis=1
    )
    rewards = np.concatenate(
        [res.results[c]["rewards"] for c in range(N_CORES)], axis=1
    )
    return next_states, rewards
